# revision 54
# baseline (speedup 1.0000x reference)
"""ARIMA(2,1,2) eps kernel: transpose-free banded matmul, bf16 HBM traffic.

The filter eps = W * y is a banded linear map (reach ~KH back / 3 fwd,
with the MA(2) IIR truncated adaptively where |h|'s tail < 1e-5). The
host pre-swizzles y per core into the exact SBUF tile image ([4 groups x
128, 16KB rows] bf16), so device loads are pure 2D copies with 16KB
contiguous lines (~382 GB/s vs 297 with 1KB lines) and the device does
ZERO PE transposes: each 128-time-step tile of y^T is the matmul
stationary ([t, b] -> out [b, n]) and the band G streams as the moving
operand. PSUM accumulation exploits the per-element has_written bit: the
first matmul in a window (start=True) clears the bank, later matmuls
overwrite-where-unset / add-where-set, so overlapping partial col-ranges
sum correctly without zero padding.

Per core: 8 batch-blocks of 128 rows, split in 2 halves of 4 so the 8
PSUM banks hold 4 blocks x 2 in-flight 512-col output windows. Window w
covers out cols [512w-3, 512w+509); tile k=4w+3 closes window w (N=128)
and opens w+1 with its spill (N=NB-128, start=True). Evacuations are
plain PSUM->SBUF bf16 copies alternating DVE/ACT (the per-column drift
bias is added on host; DVE ops with PSUM operands drop to 1x mode, so
bias-folding there cost ~15us/rep). Loads alternate the two HWDGE rings
(2MB each); stores are merged 4-window groups on SWDGE (2MB, ~4KB
lines). bf16 both ways halves HBM traffic vs f32 (16.8MB/core); f32
PSUM keeps the banded algebra exact to ~1e-7; total err ~4.6e-3 vs the
2e-2 gate. Measured: full rep ~= loads+stores with no compute interlock
-- the kernel sits on the concurrent DMA wall (~380 GB/s aggregate).
Perf decomposition modes (_CFG["mode"] in {ld,st,ldst}) kept for tuning.

Settled by A/B (back-to-back, one machine window): DMA layout 'a' (loads
alternating the two HWDGE rings, stores on SWDGE) beats single-ring
phase-separated (+11us), direction-per-ring (+9us), and split-queue
stores (+6us). For_i unroll=2 (two workload bodies per hardware-loop
iteration; `reps` = total body count) removes ~3.7us/rep of
loop-boundary serialization; unroll=4 adds nothing.
"""
import sys

for _p in ("/opt/trn_rl_repo",):
    if _p not in sys.path:
        sys.path.append(_p)

import numpy as np

B_FULL, T = 8192, 4096
N_CORES = 8
B_SH = B_FULL // N_CORES      # 1024 batch rows per core
T_OUT = T - 1                 # 4095 output cols (last 2 are zeros)
N_EPS = T - 3                 # 4093 real eps cols
P = 128
KH_MAX = 48
# adaptive IIR truncation, set by host_constants from theta's actual decay:
# tile k feeds out cols [128k-3, 128k+126+KH); NB = 129+KH rounded even
_CFG = {"KH": KH_MAX, "NB": 178}


def host_constants(phi, theta, mu):
    phi = np.asarray(phi, np.float64)
    theta = np.asarray(theta, np.float64)
    mu = float(np.asarray(mu).reshape(-1)[0])
    h48 = np.zeros(KH_MAX)
    h48[0] = 1.0
    for k in range(1, KH_MAX):
        h48[k] = -theta[0] * h48[k - 1] - (theta[1] * h48[k - 2] if k >= 2 else 0.0)
    # truncate where the remaining |h| tail is negligible (abs err < ~1e-4)
    tail = np.cumsum(np.abs(h48[::-1]))[::-1]
    below = np.nonzero(tail < 1e-5)[0]
    KH = max(8, int(below[0])) if below.size else KH_MAX
    NB = (129 + KH + 1) // 2 * 2
    _CFG["KH"], _CFG["NB"] = KH, NB

    h = h48[:KH]
    H = np.cumsum(h)
    c = {1: -phi[1], 2: -(1.0 + phi[0]), 3: 1.0}

    # shift-invariant band: w(r) = coeff of y_t in eps_n, r = n - t
    rtab = np.zeros(4096)
    for r in range(-3, KH - 1):
        v = 0.0
        for m in (1, 2, 3):
            k = r + m
            if 0 <= k < KH:
                v += c[m] * h[k]
        rtab[r + 2048] = v

    p_ = np.arange(P)[:, None]
    gpack = np.zeros((P, 2 * NB), np.float64)
    # G_band[p, j] = w(j - 3 - p); tile k feeds out cols [128k-3, 128k+126+KH)
    j_ = np.arange(NB)[None, :]
    gpack[:, NB:2 * NB] = rtab[np.clip(j_ - 3 - p_, -2048, 2047) + 2048]
    # G0: truncated-start weights for tile 0 (out cols [0,126+KH))
    n_ = np.arange(NB - 3)[None, :]
    G0 = np.zeros((P, NB - 3))
    for m in (1, 2, 3):
        k = n_ + m - p_
        valid = (k >= 0) & (k <= np.minimum(n_, KH - 1))
        G0 += c[m] * np.where(valid, h[np.clip(k, 0, KH - 1)], 0.0)
    gpack[:, 3:NB] = G0

    gpack = gpack.astype(np.float32)
    bias0 = (-mu * H[np.minimum(np.arange(512), KH - 1)]).astype(np.float32)
    bias_const = float(-mu * H[KH - 1])
    return gpack, np.broadcast_to(bias0.reshape(1, 512), (P, 512)).copy(), bias_const


def build_program(bias_const, reps=1, internal=False):
    import concourse.bacc as bacc
    import concourse.mybir as mybir
    from concourse.tile import TileContext

    f32 = mybir.dt.float32
    bf16 = mybir.dt.bfloat16
    i8 = mybir.dt.int8
    alu = mybir.AluOpType
    NB = _CFG["NB"]
    GW = 2 * NB
    # output int8 scale: device stores round(W*y * oscale); |W*y| <= B with
    # oscale = 127/B guarantees no saturation; host divides and adds bias
    oscale = float(_CFG.get("oscale", 16.0))
    mode = _CFG.get("mode", "full")   # full | ld | st | ldst (perf decomposition)
    # DMA queue layout:
    #  a: loads alternate sync/scalar rings, stores on SWDGE (gpsimd)
    #  b: everything on the sync ring -> FIFO phase-separates loads/stores
    #  c: loads on sync, stores on scalar (direction per HWDGE ring)
    #  d: loads alternate rings, stores alternate SWDGE/scalar
    layout = _CFG.get("dma", "a")

    nc = bacc.Bacc()
    g_in = nc.declare_dram_parameter("gmats", [P, GW], f32, isOutput=False)
    # y arrives host-swizzled to the SBUF tile image: row 128*(2h+u)+p,
    # col 512*kk+c  <->  y^T[2048u+128kk+p, 512h+c]; 16KB contiguous lines
    if internal:
        yio = nc.declare_dram_parameter("yio", [1, 4], f32, isOutput=True)
        y_in = nc.dram_tensor("ydr", [4 * P, 8192], bf16)
        out = nc.dram_tensor("odr", [B_SH, T_OUT], i8)
    else:
        y_in = nc.declare_dram_parameter("y", [4 * P, 8192], bf16, isOutput=False)
        out = nc.declare_dram_parameter("out", [B_SH, T_OUT], i8, isOutput=True)

    with TileContext(nc) as tc:
        with (
            tc.tile_pool(name="consts", bufs=1) as cpool,
            tc.tile_pool(name="ld", bufs=(3 if _CFG.get("ld4") else 4)) as ldpool,
            tc.tile_pool(name="st", bufs=3) as stpool,
            tc.tile_pool(name="pacc", bufs=8, space="PSUM") as pacc,
        ):
            gr = cpool.tile([P, GW], bf16)
            nc.gpsimd.dma_start(out=gr[:], in_=g_in[:])   # SWDGE f32->bf16 cast
            g0 = gr[:, 0:NB]                  # tile-0 band (3 leading zero cols)
            gb = gr[:, NB:NB + NB]            # interior band [0:NB)
            gb_close = gr[:, NB:NB + 128]     # closing slice [0:128)
            gb_spill = gr[:, NB + 128:NB + NB]  # spill slice [128:NB)

            if internal:
                zf = cpool.tile([P, 8192], bf16)
                nc.vector.memset(zf[:], 0.0)
                for i in range(4):
                    nc.sync.dma_start(out=y_in[i * P:(i + 1) * P, :], in_=zf[:])

            # store group u covers windows 4u..4u+3; per-b width and the
            # SBUF col offset of window j's evacuation within the group tile
            STW = (2045, 2050)                # g0: 509+3*512, g1: 4*512+2 zeros

            def st_off(j, b):
                u, wg = j // 4, j % 4
                if u == 0:
                    return STW[0] * b + (0 if wg == 0 else 509 + 512 * (wg - 1))
                return STW[1] * b + 512 * wg

            def ld_eng(u):
                if layout in ("b", "c"):
                    return nc.sync
                return nc.sync if u % 2 == 0 else nc.scalar

            def st_eng(idx):
                if layout == "b":
                    return nc.sync
                if layout == "c":
                    return nc.scalar
                if layout == "d":
                    return nc.gpsimd if idx % 2 == 0 else nc.scalar
                return nc.gpsimd

            if mode in ("st", "ldst"):
                stz = [cpool.tile([P, 4 * STW[1]], i8, name=f"stz{i}")
                       for i in range(2)]
                for z in stz:
                    nc.vector.memset(z[:], 0.0)

            def body_strip():
                for h in range(2):
                    if mode in ("ld", "ldst"):
                        for u in range(2):
                            ld = ldpool.tile([P, 8192], bf16, tag="ld")
                            eng = nc.sync if u % 2 == 0 else nc.scalar
                            g = 2 * h + u
                            eng.dma_start(
                                out=ld[:], in_=y_in[P * g:P * (g + 1), :])
                    if mode in ("st", "ldst"):
                        for u2 in range(2):
                            c0 = 0 if u2 == 0 else 2045
                            c1 = 2045 if u2 == 0 else 4095
                            nc.gpsimd.dma_start(
                                out=out[512 * h:512 * (h + 1), c0:c1]
                                .rearrange("(k p) c -> p k c", p=P),
                                in_=stz[u2][:, 0:4 * (c1 - c0)])

            def body():
                if mode != "full":
                    body_strip()
                    return
                for h in range(2):            # batch halves: rows [512h, 512h+512)
                    wincur = [None] * 4
                    winnext = [None] * 4
                    st = None
                    if _CFG.get("ld4"):
                        # one 4MB load per half (16KB lines x2 per partition)
                        ld = ldpool.tile([P, 16384], bf16, tag="ld")
                        lds = [ld[:, 0:8192], ld[:, 8192:16384]]
                        eng = nc.sync if h == 0 else nc.scalar
                        eng.dma_start(
                            out=ld[:],
                            in_=y_in[256 * h:256 * (h + 1), :]
                            .rearrange("(k p) c -> p k c", p=P))
                    else:
                        lds = []
                        for u in range(2):    # 2MB load groups: tiles 16u..16u+15
                            ld = ldpool.tile([P, 8192], bf16, tag="ld")
                            lds.append(ld)
                            g = 2 * h + u
                            ld_eng(u).dma_start(
                                out=ld[:], in_=y_in[P * g:P * (g + 1), :])
                    for j in range(8):        # output window index
                        u = j // 4
                        for k4 in range(4):
                            k = 4 * j + k4
                            for b in range(4):
                                lo = 512 * (k - 16 * u) + 128 * b
                                lhs = lds[u][:, lo:lo + 128]
                                if k == 0:
                                    pw = pacc.tile([P, 512], f32, tag="acc")
                                    wincur[b] = pw
                                    nc.tensor.matmul(pw[:, 0:NB], lhs, g0,
                                                     start=True, stop=False)
                                elif k4 < 3:
                                    nc.tensor.matmul(
                                        wincur[b][:, 128 * k4:128 * k4 + NB],
                                        lhs, gb, start=False, stop=False)
                                else:
                                    nc.tensor.matmul(wincur[b][:, 384:512], lhs,
                                                     gb_close, start=False, stop=True)
                                    if k < 31:
                                        pw = pacc.tile([P, 512], f32, tag="acc")
                                        winnext[b] = pw
                                        nc.tensor.matmul(pw[:, 0:NB - 128], lhs,
                                                         gb_spill,
                                                         start=True, stop=False)
                                    # evacuate window j for this b-block: plain
                                    # PSUM->SBUF bf16 copy (bias added on host)
                                    if b == 0 and j % 4 == 0:
                                        st = stpool.tile([P, 4 * STW[1]], i8,
                                                         tag="st")
                                    o0 = st_off(j, b)
                                    src = (wincur[b][:, 3:512] if j == 0
                                           else wincur[b][:])
                                    dst = st[:, o0:o0 + (509 if j == 0 else 512)]
                                    if j == 7:
                                        nc.vector.memset(
                                            st[:, o0 + 512:o0 + 514], 0.0)
                                    # DVE tensor_scalar from f32 PSUM is 1x mode
                                    # (~690ns) vs ACT ~470ns: give ACT 40/64
                                    if b == 0 or (b == 2 and j % 2 == 0):
                                        nc.vector.tensor_scalar(
                                            out=dst, in0=src,
                                            scalar1=oscale, scalar2=0.0,
                                            op0=alu.mult, op1=alu.add)
                                    else:
                                        nc.scalar.activation(
                                            out=dst, in_=src,
                                            func=mybir.ActivationFunctionType.Copy,
                                            bias=0.0, scale=oscale)
                                    wincur[b] = winnext[b]
                        if j % 4 == 3:        # store the 4-window group
                            u2 = j // 4
                            c0 = 0 if u2 == 0 else 2045
                            c1 = 2045 if u2 == 0 else 4095
                            st_eng(2 * h + u2).dma_start(
                                out=out[512 * h:512 * (h + 1), c0:c1]
                                .rearrange("(k p) c -> p k c", p=P),
                                in_=st[:, 0:4 * (c1 - c0)])

            # reps = TOTAL body count; unrolling 2 bodies per For_i iteration
            # removes ~3.7us/rep of loop-boundary serialization (measured)
            unroll = _CFG.get("unroll", 2)
            if reps == 1:
                body()
            else:
                if reps % unroll:
                    unroll = 1
                with tc.For_i(0, reps // unroll, 1) as _r:
                    for _ in range(unroll):
                        body()
            if internal:
                nc.gpsimd.dma_start(out=yio[:], in_=gr[0:1, 0:4])
    nc.finalize()
    return nc


def kernel(y, phi, theta, mu):
    import ml_dtypes
    from concourse.bass_utils import run_bass_kernel_spmd

    y = np.asarray(y, np.float32)
    assert y.shape == (B_FULL, T), y.shape
    gmats, bias0, bias_const = host_constants(phi, theta, mu)

    # int8 output scale: rigorous no-saturation bound |W*y| <= sum|w|*max|y|
    NB = _CFG["NB"]
    b_w = float(np.abs(gmats[0, NB:2 * NB]).sum())
    bound = b_w * float(np.abs(y).max()) * 1.02 + 1e-6
    oscale = 127.0 / bound
    _CFG["oscale"] = oscale

    nc = build_program(bias_const)
    in_maps = []
    for c in range(N_CORES):
        yc = y[c * B_SH:(c + 1) * B_SH].astype(ml_dtypes.bfloat16)
        # swizzle to the SBUF tile image: [h, c, u, kk, p] -> [h, u, p, kk, c]
        ysw = np.ascontiguousarray(
            yc.reshape(2, 512, 2, 16, P).transpose(0, 2, 4, 3, 1)
        ).reshape(4 * P, 8192)
        in_maps.append({"y": ysw, "gmats": gmats})
    res = run_bass_kernel_spmd(nc, in_maps, list(range(N_CORES)))
    out = np.concatenate(
        [res.results[c]["out"].astype(np.float32) for c in range(N_CORES)],
        axis=0)
    out *= np.float32(1.0 / oscale)   # undo the int8 quantization scale
    # per-column drift bias applied host-side (device stores pure W*y)
    bias_full = np.full(T_OUT, bias_const, np.float32)
    bias_full[:512] = bias0[0]
    bias_full[N_EPS:] = 0.0
    out += bias_full[None, :]
    return out


# revision 57
# speedup vs baseline: 1.1000x; 1.1000x over previous
"""ARIMA(2,1,2) eps kernel: transpose-free banded matmul, bf16 HBM traffic.

The filter eps = W * y is a banded linear map (reach ~KH back / 3 fwd,
with the MA(2) IIR truncated adaptively where |h|'s tail < 1e-5). The
host pre-swizzles y per core into the exact SBUF tile image ([4 groups x
128, 16KB rows] bf16), so device loads are pure 2D copies with 16KB
contiguous lines (~382 GB/s vs 297 with 1KB lines) and the device does
ZERO PE transposes: each 128-time-step tile of y^T is the matmul
stationary ([t, b] -> out [b, n]) and the band G streams as the moving
operand. PSUM accumulation exploits the per-element has_written bit: the
first matmul in a window (start=True) clears the bank, later matmuls
overwrite-where-unset / add-where-set, so overlapping partial col-ranges
sum correctly without zero padding.

Per core: 8 batch-blocks of 128 rows, split in 2 halves of 4 so the 8
PSUM banks hold 4 blocks x 2 in-flight 512-col output windows. Window w
covers out cols [512w-3, 512w+509); tile k=4w+3 closes window w (N=128)
and opens w+1 with its spill (N=NB-128, start=True). Evacuations are
plain PSUM->SBUF bf16 copies alternating DVE/ACT (the per-column drift
bias is added on host; DVE ops with PSUM operands drop to 1x mode, so
bias-folding there cost ~15us/rep). Loads alternate the two HWDGE rings
(2MB each); stores are merged 4-window groups on SWDGE (2MB, ~4KB
lines). bf16 both ways halves HBM traffic vs f32 (16.8MB/core); f32
PSUM keeps the banded algebra exact to ~1e-7; total err ~4.6e-3 vs the
2e-2 gate. Measured: full rep ~= loads+stores with no compute interlock
-- the kernel sits on the concurrent DMA wall (~380 GB/s aggregate).
Perf decomposition modes (_CFG["mode"] in {ld,st,ldst}) kept for tuning.

Settled by A/B (back-to-back, one machine window): DMA layout 'a' (loads
alternating the two HWDGE rings, stores on SWDGE) beats single-ring
phase-separated (+11us), direction-per-ring (+9us), and split-queue
stores (+6us). For_i unroll=2 (two workload bodies per hardware-loop
iteration; `reps` = total body count) removes ~3.7us/rep of
loop-boundary serialization; unroll=4 adds nothing.
"""
import sys

for _p in ("/opt/trn_rl_repo",):
    if _p not in sys.path:
        sys.path.append(_p)

import numpy as np

B_FULL, T = 8192, 4096
N_CORES = 8
B_SH = B_FULL // N_CORES      # 1024 batch rows per core
T_OUT = T - 1                 # 4095 output cols (last 2 are zeros)
N_EPS = T - 3                 # 4093 real eps cols
P = 128
KH_MAX = 48
# adaptive IIR truncation, set by host_constants from theta's actual decay:
# tile k feeds out cols [128k-3, 128k+126+KH); NB = 129+KH rounded even
_CFG = {"KH": KH_MAX, "NB": 178}


def host_constants(phi, theta, mu):
    phi = np.asarray(phi, np.float64)
    theta = np.asarray(theta, np.float64)
    mu = float(np.asarray(mu).reshape(-1)[0])
    h48 = np.zeros(KH_MAX)
    h48[0] = 1.0
    for k in range(1, KH_MAX):
        h48[k] = -theta[0] * h48[k - 1] - (theta[1] * h48[k - 2] if k >= 2 else 0.0)
    # truncate where the remaining |h| tail is negligible (abs err < ~1e-4)
    tail = np.cumsum(np.abs(h48[::-1]))[::-1]
    below = np.nonzero(tail < 1e-5)[0]
    KH = max(8, int(below[0])) if below.size else KH_MAX
    NB = (129 + KH + 1) // 2 * 2
    _CFG["KH"], _CFG["NB"] = KH, NB

    h = h48[:KH]
    H = np.cumsum(h)
    c = {1: -phi[1], 2: -(1.0 + phi[0]), 3: 1.0}

    # shift-invariant band: w(r) = coeff of y_t in eps_n, r = n - t
    rtab = np.zeros(4096)
    for r in range(-3, KH - 1):
        v = 0.0
        for m in (1, 2, 3):
            k = r + m
            if 0 <= k < KH:
                v += c[m] * h[k]
        rtab[r + 2048] = v

    p_ = np.arange(P)[:, None]
    gpack = np.zeros((P, 2 * NB), np.float64)
    # G_band[p, j] = w(j - 3 - p); tile k feeds out cols [128k-3, 128k+126+KH)
    j_ = np.arange(NB)[None, :]
    gpack[:, NB:2 * NB] = rtab[np.clip(j_ - 3 - p_, -2048, 2047) + 2048]
    # G0: truncated-start weights for tile 0 (out cols [0,126+KH))
    n_ = np.arange(NB - 3)[None, :]
    G0 = np.zeros((P, NB - 3))
    for m in (1, 2, 3):
        k = n_ + m - p_
        valid = (k >= 0) & (k <= np.minimum(n_, KH - 1))
        G0 += c[m] * np.where(valid, h[np.clip(k, 0, KH - 1)], 0.0)
    gpack[:, 3:NB] = G0

    gpack = gpack.astype(np.float32)
    bias0 = (-mu * H[np.minimum(np.arange(512), KH - 1)]).astype(np.float32)
    bias_const = float(-mu * H[KH - 1])
    return gpack, np.broadcast_to(bias0.reshape(1, 512), (P, 512)).copy(), bias_const


def build_program(bias_const, reps=1, internal=False):
    import concourse.bacc as bacc
    import concourse.mybir as mybir
    from concourse.tile import TileContext

    f32 = mybir.dt.float32
    bf16 = mybir.dt.bfloat16
    i8 = mybir.dt.int8
    alu = mybir.AluOpType
    NB = _CFG["NB"]
    GW = 2 * NB
    # output int8 scale: device stores round(W*y * oscale); |W*y| <= B with
    # oscale = 127/B guarantees no saturation; host divides and adds bias
    oscale = float(_CFG.get("oscale", 16.0))
    mode = _CFG.get("mode", "full")   # full | ld | st | ldst (perf decomposition)
    # DMA queue layout:
    #  a: loads alternate sync/scalar rings, stores on SWDGE (gpsimd)
    #  b: everything on the sync ring -> FIFO phase-separates loads/stores
    #  c: loads on sync, stores on scalar (direction per HWDGE ring)
    #  d: loads alternate rings, stores alternate SWDGE/scalar
    layout = _CFG.get("dma", "a")

    nc = bacc.Bacc()
    g_in = nc.declare_dram_parameter("gmats", [P, GW], f32, isOutput=False)
    # y arrives host-swizzled to the SBUF tile image: row 128*(2h+u)+p,
    # col 512*kk+c  <->  y^T[2048u+128kk+p, 512h+c]; 16KB contiguous lines
    if internal:
        yio = nc.declare_dram_parameter("yio", [1, 4], f32, isOutput=True)
        y_in = nc.dram_tensor("ydr", [4 * P, 8192], bf16)
        out = nc.dram_tensor("odr", [B_SH, T_OUT], i8)
    else:
        y_in = nc.declare_dram_parameter("y", [4 * P, 8192], bf16, isOutput=False)
        out = nc.declare_dram_parameter("out", [B_SH, T_OUT], i8, isOutput=True)

    with TileContext(nc) as tc:
        with (
            tc.tile_pool(name="consts", bufs=1) as cpool,
            tc.tile_pool(name="ld", bufs=(3 if _CFG.get("ld4") else 4)) as ldpool,
            tc.tile_pool(name="st", bufs=3) as stpool,
            tc.tile_pool(name="pacc", bufs=8, space="PSUM") as pacc,
        ):
            gr = cpool.tile([P, GW], bf16)
            nc.gpsimd.dma_start(out=gr[:], in_=g_in[:])   # SWDGE f32->bf16 cast
            g0 = gr[:, 0:NB]                  # tile-0 band (3 leading zero cols)
            gb = gr[:, NB:NB + NB]            # interior band [0:NB)
            gb_close = gr[:, NB:NB + 128]     # closing slice [0:128)
            gb_spill = gr[:, NB + 128:NB + NB]  # spill slice [128:NB)

            if internal:
                zf = cpool.tile([P, 8192], bf16)
                nc.vector.memset(zf[:], 0.0)
                for i in range(4):
                    nc.sync.dma_start(out=y_in[i * P:(i + 1) * P, :], in_=zf[:])

            # store group u covers windows 4u..4u+3; per-b width and the
            # SBUF col offset of window j's evacuation within the group tile
            STW = (2045, 2050)                # g0: 509+3*512, g1: 4*512+2 zeros

            def st_off(j, b):
                u, wg = j // 4, j % 4
                if u == 0:
                    return STW[0] * b + (0 if wg == 0 else 509 + 512 * (wg - 1))
                return STW[1] * b + 512 * wg

            def ld_eng(u):
                if layout in ("b", "c"):
                    return nc.sync
                return nc.sync if u % 2 == 0 else nc.scalar

            def st_eng(idx):
                if layout == "b":
                    return nc.sync
                if layout == "c":
                    return nc.scalar
                if layout == "d":
                    return nc.gpsimd if idx % 2 == 0 else nc.scalar
                return nc.gpsimd

            if mode in ("st", "ldst"):
                stz = [cpool.tile([P, 4 * STW[1]], i8, name=f"stz{i}")
                       for i in range(2)]
                for z in stz:
                    nc.vector.memset(z[:], 0.0)

            def body_strip():
                for h in range(2):
                    if mode in ("ld", "ldst"):
                        for u in range(2):
                            ld = ldpool.tile([P, 8192], bf16, tag="ld")
                            eng = nc.sync if u % 2 == 0 else nc.scalar
                            g = 2 * h + u
                            eng.dma_start(
                                out=ld[:], in_=y_in[P * g:P * (g + 1), :])
                    if mode in ("st", "ldst"):
                        for u2 in range(2):
                            c0 = 0 if u2 == 0 else 2045
                            c1 = 2045 if u2 == 0 else 4095
                            nc.gpsimd.dma_start(
                                out=out[512 * h:512 * (h + 1), c0:c1]
                                .rearrange("(k p) c -> p k c", p=P),
                                in_=stz[u2][:, 0:4 * (c1 - c0)])

            def body():
                if mode != "full":
                    body_strip()
                    return
                for h in range(2):            # batch halves: rows [512h, 512h+512)
                    wincur = [None] * 4
                    winnext = [None] * 4
                    st = None
                    if _CFG.get("ld4"):
                        # one 4MB load per half (16KB lines x2 per partition)
                        ld = ldpool.tile([P, 16384], bf16, tag="ld")
                        lds = [ld[:, 0:8192], ld[:, 8192:16384]]
                        eng = nc.sync if h == 0 else nc.scalar
                        eng.dma_start(
                            out=ld[:],
                            in_=y_in[256 * h:256 * (h + 1), :]
                            .rearrange("(k p) c -> p k c", p=P))
                    else:
                        lds = []
                        for u in range(2):    # 2MB load groups: tiles 16u..16u+15
                            ld = ldpool.tile([P, 8192], bf16, tag="ld")
                            lds.append(ld)
                            g = 2 * h + u
                            ld_eng(u).dma_start(
                                out=ld[:], in_=y_in[P * g:P * (g + 1), :])
                    for j in range(8):        # output window index
                        u = j // 4
                        for k4 in range(4):
                            k = 4 * j + k4
                            for b in range(4):
                                lo = 512 * (k - 16 * u) + 128 * b
                                lhs = lds[u][:, lo:lo + 128]
                                if k == 0:
                                    pw = pacc.tile([P, 512], f32, tag="acc")
                                    wincur[b] = pw
                                    nc.tensor.matmul(pw[:, 0:NB], lhs, g0,
                                                     start=True, stop=False)
                                elif k4 < 3:
                                    nc.tensor.matmul(
                                        wincur[b][:, 128 * k4:128 * k4 + NB],
                                        lhs, gb, start=False, stop=False)
                                else:
                                    nc.tensor.matmul(wincur[b][:, 384:512], lhs,
                                                     gb_close, start=False, stop=True)
                                    if k < 31:
                                        pw = pacc.tile([P, 512], f32, tag="acc")
                                        winnext[b] = pw
                                        nc.tensor.matmul(pw[:, 0:NB - 128], lhs,
                                                         gb_spill,
                                                         start=True, stop=False)
                                    # evacuate window j for this b-block: plain
                                    # PSUM->SBUF bf16 copy (bias added on host)
                                    if b == 0 and j % 4 == 0:
                                        st = stpool.tile([P, 4 * STW[1]], i8,
                                                         tag="st")
                                    o0 = st_off(j, b)
                                    src = (wincur[b][:, 3:512] if j == 0
                                           else wincur[b][:])
                                    dst = st[:, o0:o0 + (509 if j == 0 else 512)]
                                    if j == 7:
                                        nc.vector.memset(
                                            st[:, o0 + 512:o0 + 514], 0.0)
                                    # scaled f32 PSUM -> int8 SBUF evacuation,
                                    # split evenly DVE/ACT (24/40 split measured worse)
                                    ev = _CFG.get("evac", "mix")
                                    if ev == "dve" or (ev == "mix" and b % 2 == 0):
                                        nc.vector.tensor_scalar(
                                            out=dst, in0=src,
                                            scalar1=oscale, scalar2=0.0,
                                            op0=alu.mult, op1=alu.add)
                                    else:
                                        nc.scalar.activation(
                                            out=dst, in_=src,
                                            func=mybir.ActivationFunctionType.Copy,
                                            bias=0.0, scale=oscale)
                                    wincur[b] = winnext[b]
                        if j % 4 == 3:        # store the 4-window group
                            u2 = j // 4
                            c0 = 0 if u2 == 0 else 2045
                            c1 = 2045 if u2 == 0 else 4095
                            st_eng(2 * h + u2).dma_start(
                                out=out[512 * h:512 * (h + 1), c0:c1]
                                .rearrange("(k p) c -> p k c", p=P),
                                in_=st[:, 0:4 * (c1 - c0)])

            # reps = TOTAL body count; unrolling 2 bodies per For_i iteration
            # removes ~3.7us/rep of loop-boundary serialization (measured)
            unroll = _CFG.get("unroll", 2)
            if reps == 1:
                body()
            else:
                if reps % unroll:
                    unroll = 1
                with tc.For_i(0, reps // unroll, 1) as _r:
                    for _ in range(unroll):
                        body()
            if internal:
                nc.gpsimd.dma_start(out=yio[:], in_=gr[0:1, 0:4])
    nc.finalize()
    return nc


def kernel(y, phi, theta, mu):
    import ml_dtypes
    from concourse.bass_utils import run_bass_kernel_spmd

    y = np.asarray(y, np.float32)
    assert y.shape == (B_FULL, T), y.shape
    gmats, bias0, bias_const = host_constants(phi, theta, mu)

    # int8 output scale: rigorous no-saturation bound |W*y| <= sum|w|*max|y|
    NB = _CFG["NB"]
    b_w = float(np.abs(gmats[0, NB:2 * NB]).sum())
    bound = b_w * float(np.abs(y).max()) * 1.02 + 1e-6
    oscale = 127.0 / bound
    _CFG["oscale"] = oscale

    nc = build_program(bias_const)
    in_maps = []
    for c in range(N_CORES):
        yc = y[c * B_SH:(c + 1) * B_SH].astype(ml_dtypes.bfloat16)
        # swizzle to the SBUF tile image: [h, c, u, kk, p] -> [h, u, p, kk, c]
        ysw = np.ascontiguousarray(
            yc.reshape(2, 512, 2, 16, P).transpose(0, 2, 4, 3, 1)
        ).reshape(4 * P, 8192)
        in_maps.append({"y": ysw, "gmats": gmats})
    res = run_bass_kernel_spmd(nc, in_maps, list(range(N_CORES)))
    out = np.concatenate(
        [res.results[c]["out"].astype(np.float32) for c in range(N_CORES)],
        axis=0)
    out *= np.float32(1.0 / oscale)   # undo the int8 quantization scale
    # per-column drift bias applied host-side (device stores pure W*y)
    bias_full = np.full(T_OUT, bias_const, np.float32)
    bias_full[:512] = bias0[0]
    bias_full[N_EPS:] = 0.0
    out += bias_full[None, :]
    return out


# revision 59
# speedup vs baseline: 1.1147x; 1.0134x over previous
"""ARIMA(2,1,2) eps kernel: transpose-free banded matmul, bf16 HBM traffic.

The filter eps = W * y is a banded linear map (reach ~KH back / 3 fwd,
with the MA(2) IIR truncated adaptively where |h|'s tail < 1e-5). The
host pre-swizzles y per core into the exact SBUF tile image ([4 groups x
128, 16KB rows] bf16), so device loads are pure 2D copies with 16KB
contiguous lines (~382 GB/s vs 297 with 1KB lines) and the device does
ZERO PE transposes: each 128-time-step tile of y^T is the matmul
stationary ([t, b] -> out [b, n]) and the band G streams as the moving
operand. PSUM accumulation exploits the per-element has_written bit: the
first matmul in a window (start=True) clears the bank, later matmuls
overwrite-where-unset / add-where-set, so overlapping partial col-ranges
sum correctly without zero padding.

Per core: 8 batch-blocks of 128 rows, split in 2 halves of 4 so the 8
PSUM banks hold 4 blocks x 2 in-flight 512-col output windows. Window w
covers out cols [512w-3, 512w+509); tile k=4w+3 closes window w (N=128)
and opens w+1 with its spill (N=NB-128, start=True). Evacuations are
plain PSUM->SBUF bf16 copies alternating DVE/ACT (the per-column drift
bias is added on host; DVE ops with PSUM operands drop to 1x mode, so
bias-folding there cost ~15us/rep). Loads alternate the two HWDGE rings
(2MB each); stores are merged 4-window groups on SWDGE (2MB, ~4KB
lines). bf16 both ways halves HBM traffic vs f32 (16.8MB/core); f32
PSUM keeps the banded algebra exact to ~1e-7; total err ~4.6e-3 vs the
2e-2 gate. Measured: full rep ~= loads+stores with no compute interlock
-- the kernel sits on the concurrent DMA wall (~380 GB/s aggregate).
Perf decomposition modes (_CFG["mode"] in {ld,st,ldst}) kept for tuning.

Settled by A/B (back-to-back, one machine window): DMA layout 'a' (loads
alternating the two HWDGE rings, stores on SWDGE) beats single-ring
phase-separated (+11us), direction-per-ring (+9us), and split-queue
stores (+6us). For_i unroll=2 (two workload bodies per hardware-loop
iteration; `reps` = total body count) removes ~3.7us/rep of
loop-boundary serialization; unroll=4 adds nothing.
"""
import sys

for _p in ("/opt/trn_rl_repo",):
    if _p not in sys.path:
        sys.path.append(_p)

import numpy as np

B_FULL, T = 8192, 4096
N_CORES = 8
B_SH = B_FULL // N_CORES      # 1024 batch rows per core
T_OUT = T - 1                 # 4095 output cols (last 2 are zeros)
N_EPS = T - 3                 # 4093 real eps cols
P = 128
KH_MAX = 48
# adaptive IIR truncation, set by host_constants from theta's actual decay:
# tile k feeds out cols [128k-3, 128k+126+KH); NB = 129+KH rounded even
_CFG = {"KH": KH_MAX, "NB": 178}


def host_constants(phi, theta, mu):
    phi = np.asarray(phi, np.float64)
    theta = np.asarray(theta, np.float64)
    mu = float(np.asarray(mu).reshape(-1)[0])
    h48 = np.zeros(KH_MAX)
    h48[0] = 1.0
    for k in range(1, KH_MAX):
        h48[k] = -theta[0] * h48[k - 1] - (theta[1] * h48[k - 2] if k >= 2 else 0.0)
    # truncate where the remaining |h| tail is negligible (abs err < ~1e-4)
    tail = np.cumsum(np.abs(h48[::-1]))[::-1]
    below = np.nonzero(tail < 1e-5)[0]
    KH = max(8, int(below[0])) if below.size else KH_MAX
    NB = (129 + KH + 1) // 2 * 2
    _CFG["KH"], _CFG["NB"] = KH, NB

    h = h48[:KH]
    H = np.cumsum(h)
    c = {1: -phi[1], 2: -(1.0 + phi[0]), 3: 1.0}

    # shift-invariant band: w(r) = coeff of y_t in eps_n, r = n - t
    rtab = np.zeros(4096)
    for r in range(-3, KH - 1):
        v = 0.0
        for m in (1, 2, 3):
            k = r + m
            if 0 <= k < KH:
                v += c[m] * h[k]
        rtab[r + 2048] = v

    p_ = np.arange(P)[:, None]
    gpack = np.zeros((P, 2 * NB), np.float64)
    # G_band[p, j] = w(j - 3 - p); tile k feeds out cols [128k-3, 128k+126+KH)
    j_ = np.arange(NB)[None, :]
    gpack[:, NB:2 * NB] = rtab[np.clip(j_ - 3 - p_, -2048, 2047) + 2048]
    # G0: truncated-start weights for tile 0 (out cols [0,126+KH))
    n_ = np.arange(NB - 3)[None, :]
    G0 = np.zeros((P, NB - 3))
    for m in (1, 2, 3):
        k = n_ + m - p_
        valid = (k >= 0) & (k <= np.minimum(n_, KH - 1))
        G0 += c[m] * np.where(valid, h[np.clip(k, 0, KH - 1)], 0.0)
    gpack[:, 3:NB] = G0

    gpack = gpack.astype(np.float32)
    bias0 = (-mu * H[np.minimum(np.arange(512), KH - 1)]).astype(np.float32)
    bias_const = float(-mu * H[KH - 1])
    return gpack, np.broadcast_to(bias0.reshape(1, 512), (P, 512)).copy(), bias_const


def build_program(bias_const, reps=1, internal=False):
    import concourse.bacc as bacc
    import concourse.mybir as mybir
    from concourse.tile import TileContext

    f32 = mybir.dt.float32
    bf16 = mybir.dt.bfloat16
    i8 = mybir.dt.int8
    alu = mybir.AluOpType
    NB = _CFG["NB"]
    GW = 2 * NB
    # output int8 scale: device stores round(W*y * oscale); |W*y| <= B with
    # oscale = 127/B guarantees no saturation; host divides and adds bias
    oscale = float(_CFG.get("oscale", 16.0))
    # bf16 PSUM accumulation: half-bank windows -> 16 rotation slots and
    # 2x-mode evacuation (16-bit PSUM operand); one extra bf16 rounding
    pdt = bf16 if _CFG.get("psum16", False) else f32
    mode = _CFG.get("mode", "full")   # full | ld | st | ldst (perf decomposition)
    # DMA queue layout:
    #  a: loads alternate sync/scalar rings, stores on SWDGE (gpsimd)
    #  b: everything on the sync ring -> FIFO phase-separates loads/stores
    #  c: loads on sync, stores on scalar (direction per HWDGE ring)
    #  d: loads alternate rings, stores alternate SWDGE/scalar
    layout = _CFG.get("dma", "a")

    nc = bacc.Bacc()
    g_in = nc.declare_dram_parameter("gmats", [P, GW], f32, isOutput=False)
    # y arrives host-swizzled to the SBUF tile image: row 128*(2h+u)+p,
    # col 512*kk+c  <->  y^T[2048u+128kk+p, 512h+c]; 16KB contiguous lines
    if internal:
        yio = nc.declare_dram_parameter("yio", [1, 4], f32, isOutput=True)
        y_in = nc.dram_tensor("ydr", [4 * P, 8192], bf16)
        out = nc.dram_tensor("odr", [B_SH, T_OUT], i8)
    else:
        y_in = nc.declare_dram_parameter("y", [4 * P, 8192], bf16, isOutput=False)
        out = nc.declare_dram_parameter("out", [B_SH, T_OUT], i8, isOutput=True)

    with TileContext(nc) as tc:
        with (
            tc.tile_pool(name="consts", bufs=1) as cpool,
            tc.tile_pool(name="ld", bufs=(3 if _CFG.get("ld4") else 4)) as ldpool,
            tc.tile_pool(name="st", bufs=3) as stpool,
            tc.tile_pool(name="pacc",
                         bufs=(16 if _CFG.get("psum16", False) else 8),
                         space="PSUM") as pacc,
        ):
            gr = cpool.tile([P, GW], bf16)
            nc.gpsimd.dma_start(out=gr[:], in_=g_in[:])   # SWDGE f32->bf16 cast
            g0 = gr[:, 0:NB]                  # tile-0 band (3 leading zero cols)
            gb = gr[:, NB:NB + NB]            # interior band [0:NB)
            gb_close = gr[:, NB:NB + 128]     # closing slice [0:128)
            gb_spill = gr[:, NB + 128:NB + NB]  # spill slice [128:NB)

            if internal:
                zf = cpool.tile([P, 8192], bf16)
                nc.vector.memset(zf[:], 0.0)
                for i in range(4):
                    nc.sync.dma_start(out=y_in[i * P:(i + 1) * P, :], in_=zf[:])

            # store group u covers windows 4u..4u+3; per-b width and the
            # SBUF col offset of window j's evacuation within the group tile
            STW = (2045, 2050)                # g0: 509+3*512, g1: 4*512+2 zeros

            def st_off(j, b):
                u, wg = j // 4, j % 4
                if u == 0:
                    return STW[0] * b + (0 if wg == 0 else 509 + 512 * (wg - 1))
                return STW[1] * b + 512 * wg

            def ld_eng(u):
                if layout in ("b", "c"):
                    return nc.sync
                return nc.sync if u % 2 == 0 else nc.scalar

            def st_eng(idx):
                if layout == "b":
                    return nc.sync
                if layout == "c":
                    return nc.scalar
                if layout == "d":
                    return nc.gpsimd if idx % 2 == 0 else nc.scalar
                return nc.gpsimd

            if mode in ("st", "ldst"):
                stz = [cpool.tile([P, 4 * STW[1]], i8, name=f"stz{i}")
                       for i in range(2)]
                for z in stz:
                    nc.vector.memset(z[:], 0.0)

            def body_strip():
                for h in range(2):
                    if mode in ("ld", "ldst"):
                        for u in range(2):
                            ld = ldpool.tile([P, 8192], bf16, tag="ld")
                            eng = nc.sync if u % 2 == 0 else nc.scalar
                            g = 2 * h + u
                            eng.dma_start(
                                out=ld[:], in_=y_in[P * g:P * (g + 1), :])
                    if mode in ("st", "ldst"):
                        for u2 in range(2):
                            c0 = 0 if u2 == 0 else 2045
                            c1 = 2045 if u2 == 0 else 4095
                            nc.gpsimd.dma_start(
                                out=out[512 * h:512 * (h + 1), c0:c1]
                                .rearrange("(k p) c -> p k c", p=P),
                                in_=stz[u2][:, 0:4 * (c1 - c0)])

            def body():
                if mode != "full":
                    body_strip()
                    return
                for h in range(2):            # batch halves: rows [512h, 512h+512)
                    wincur = [None] * 4
                    winnext = [None] * 4
                    st = None
                    if _CFG.get("ld4"):
                        # one 4MB load per half (16KB lines x2 per partition)
                        ld = ldpool.tile([P, 16384], bf16, tag="ld")
                        lds = [ld[:, 0:8192], ld[:, 8192:16384]]
                        eng = nc.sync if h == 0 else nc.scalar
                        eng.dma_start(
                            out=ld[:],
                            in_=y_in[256 * h:256 * (h + 1), :]
                            .rearrange("(k p) c -> p k c", p=P))
                    else:
                        lds = []
                        for u in range(2):    # 2MB load groups: tiles 16u..16u+15
                            ld = ldpool.tile([P, 8192], bf16, tag="ld")
                            lds.append(ld)
                            g = 2 * h + u
                            ld_eng(u).dma_start(
                                out=ld[:], in_=y_in[P * g:P * (g + 1), :])
                    for j in range(8):        # output window index
                        u = j // 4
                        for k4 in range(4):
                            k = 4 * j + k4
                            for b in range(4):
                                lo = 512 * (k - 16 * u) + 128 * b
                                lhs = lds[u][:, lo:lo + 128]
                                if k == 0:
                                    pw = pacc.tile([P, 512], pdt, tag="acc")
                                    wincur[b] = pw
                                    nc.tensor.matmul(pw[:, 0:NB], lhs, g0,
                                                     start=True, stop=False)
                                elif k4 < 3:
                                    nc.tensor.matmul(
                                        wincur[b][:, 128 * k4:128 * k4 + NB],
                                        lhs, gb, start=False, stop=False)
                                else:
                                    nc.tensor.matmul(wincur[b][:, 384:512], lhs,
                                                     gb_close, start=False, stop=True)
                                    if k < 31:
                                        pw = pacc.tile([P, 512], pdt, tag="acc")
                                        winnext[b] = pw
                                        nc.tensor.matmul(pw[:, 0:NB - 128], lhs,
                                                         gb_spill,
                                                         start=True, stop=False)
                                    # evacuate window j for this b-block: plain
                                    # PSUM->SBUF bf16 copy (bias added on host)
                                    if b == 0 and j % 4 == 0:
                                        st = stpool.tile([P, 4 * STW[1]], i8,
                                                         tag="st")
                                    o0 = st_off(j, b)
                                    src = (wincur[b][:, 3:512] if j == 0
                                           else wincur[b][:])
                                    dst = st[:, o0:o0 + (509 if j == 0 else 512)]
                                    if j == 7:
                                        nc.vector.memset(
                                            st[:, o0 + 512:o0 + 514], 0.0)
                                    # scaled f32 PSUM -> int8 SBUF evacuation,
                                    # split evenly DVE/ACT (24/40 split measured worse)
                                    ev = _CFG.get("evac", "mix")
                                    if ev == "dve" or (ev == "mix" and b % 2 == 0):
                                        nc.vector.tensor_scalar(
                                            out=dst, in0=src,
                                            scalar1=oscale, scalar2=0.0,
                                            op0=alu.mult, op1=alu.add)
                                    else:
                                        nc.scalar.activation(
                                            out=dst, in_=src,
                                            func=mybir.ActivationFunctionType.Copy,
                                            bias=0.0, scale=oscale)
                                    wincur[b] = winnext[b]
                        if j % 4 == 3:        # store the 4-window group
                            u2 = j // 4
                            c0 = 0 if u2 == 0 else 2045
                            c1 = 2045 if u2 == 0 else 4095
                            st_eng(2 * h + u2).dma_start(
                                out=out[512 * h:512 * (h + 1), c0:c1]
                                .rearrange("(k p) c -> p k c", p=P),
                                in_=st[:, 0:4 * (c1 - c0)])

            # reps = TOTAL body count; unrolling 2 bodies per For_i iteration
            # removes ~3.7us/rep of loop-boundary serialization (measured)
            unroll = _CFG.get("unroll", 2)
            if reps == 1:
                body()
            else:
                if reps % unroll:
                    unroll = 1
                with tc.For_i(0, reps // unroll, 1) as _r:
                    for _ in range(unroll):
                        body()
            if internal:
                nc.gpsimd.dma_start(out=yio[:], in_=gr[0:1, 0:4])
    nc.finalize()
    return nc


def kernel(y, phi, theta, mu):
    import ml_dtypes
    from concourse.bass_utils import run_bass_kernel_spmd

    y = np.asarray(y, np.float32)
    assert y.shape == (B_FULL, T), y.shape
    gmats, bias0, bias_const = host_constants(phi, theta, mu)

    # int8 output scale: rigorous no-saturation bound |W*y| <= sum|w|*max|y|
    NB = _CFG["NB"]
    b_w = float(np.abs(gmats[0, NB:2 * NB]).sum())
    bound = b_w * float(np.abs(y).max()) * 1.02 + 1e-6
    oscale = 127.0 / bound
    _CFG["oscale"] = oscale

    nc = build_program(bias_const)
    in_maps = []
    for c in range(N_CORES):
        yc = y[c * B_SH:(c + 1) * B_SH].astype(ml_dtypes.bfloat16)
        # swizzle to the SBUF tile image: [h, c, u, kk, p] -> [h, u, p, kk, c]
        ysw = np.ascontiguousarray(
            yc.reshape(2, 512, 2, 16, P).transpose(0, 2, 4, 3, 1)
        ).reshape(4 * P, 8192)
        in_maps.append({"y": ysw, "gmats": gmats})
    res = run_bass_kernel_spmd(nc, in_maps, list(range(N_CORES)))
    out = np.concatenate(
        [res.results[c]["out"].astype(np.float32) for c in range(N_CORES)],
        axis=0)
    out *= np.float32(1.0 / oscale)   # undo the int8 quantization scale
    # per-column drift bias applied host-side (device stores pure W*y)
    bias_full = np.full(T_OUT, bias_const, np.float32)
    bias_full[:512] = bias0[0]
    bias_full[N_EPS:] = 0.0
    out += bias_full[None, :]
    return out


# revision 61
# speedup vs baseline: 1.1486x; 1.0305x over previous
"""ARIMA(2,1,2) eps kernel: transpose-free banded matmul, bf16 HBM traffic.

The filter eps = W * y is a banded linear map (reach ~KH back / 3 fwd,
with the MA(2) IIR truncated adaptively where |h|'s tail < 1e-5). The
host pre-swizzles y per core into the exact SBUF tile image ([4 groups x
128, 16KB rows] bf16), so device loads are pure 2D copies with 16KB
contiguous lines (~382 GB/s vs 297 with 1KB lines) and the device does
ZERO PE transposes: each 128-time-step tile of y^T is the matmul
stationary ([t, b] -> out [b, n]) and the band G streams as the moving
operand. PSUM accumulation exploits the per-element has_written bit: the
first matmul in a window (start=True) clears the bank, later matmuls
overwrite-where-unset / add-where-set, so overlapping partial col-ranges
sum correctly without zero padding.

Per core: 8 batch-blocks of 128 rows, split in 2 halves of 4 so the 8
PSUM banks hold 4 blocks x 2 in-flight 512-col output windows. Window w
covers out cols [512w-3, 512w+509); tile k=4w+3 closes window w (N=128)
and opens w+1 with its spill (N=NB-128, start=True). Evacuations scale
by 127/B (B = sum|w|*max|y|, a rigorous no-saturation bound computed on
host) and cast f32 PSUM -> int8 SBUF, split evenly DVE/ACT -- the gate
is max-abs so affine int8 output passes where fp8 (relative error)
cannot; the host divides by the scale and adds the per-column drift
bias. Loads alternate the two HWDGE rings (2MB each); stores are merged
4-window int8 groups on SWDGE (~1MB). Traffic: 8.4MB bf16 loads + 4.2MB
int8 stores per core. f32 PSUM keeps the banded algebra exact to ~1e-7;
total err ~8.2e-3 vs the 2e-2 gate.
Perf decomposition modes (_CFG["mode"] in {ld,st,ldst}) kept for tuning.

Settled by A/B (back-to-back, one machine window): DMA layout 'a' (loads
alternating the two HWDGE rings, stores on SWDGE) beats single-ring
phase-separated (+11us), direction-per-ring (+9us), and split-queue
stores (+6us). For_i unroll=2 (two workload bodies per hardware-loop
iteration; `reps` = total body count) removes ~3.7us/rep of
loop-boundary serialization; unroll=4 adds nothing. Evacuation engine
probes: even DVE/ACT split optimal (all-DVE +8us, all-ACT +5.7us,
24/40 split worse); bf16 PSUM accumulation (2x evac mode + 16 window
slots) is impossible -- bass asserts matmul output must be fp32.
~41us is the floor for this decomposition: max(22us load stream, 22us
two-engine 1x-mode PSUM evacuation, ~17us PE) through an 8-bank PSUM
rotation with zero spare banks.
"""
import sys

for _p in ("/opt/trn_rl_repo",):
    if _p not in sys.path:
        sys.path.append(_p)

import numpy as np

B_FULL, T = 8192, 4096
N_CORES = 8
B_SH = B_FULL // N_CORES      # 1024 batch rows per core
T_OUT = T - 1                 # 4095 output cols (last 2 are zeros)
N_EPS = T - 3                 # 4093 real eps cols
P = 128
KH_MAX = 48
# adaptive IIR truncation, set by host_constants from theta's actual decay:
# tile k feeds out cols [128k-3, 128k+126+KH); NB = 129+KH rounded even
_CFG = {"KH": KH_MAX, "NB": 178}


def host_constants(phi, theta, mu):
    phi = np.asarray(phi, np.float64)
    theta = np.asarray(theta, np.float64)
    mu = float(np.asarray(mu).reshape(-1)[0])
    h48 = np.zeros(KH_MAX)
    h48[0] = 1.0
    for k in range(1, KH_MAX):
        h48[k] = -theta[0] * h48[k - 1] - (theta[1] * h48[k - 2] if k >= 2 else 0.0)
    # truncate where the remaining |h| tail is negligible (abs err < ~1e-4)
    tail = np.cumsum(np.abs(h48[::-1]))[::-1]
    below = np.nonzero(tail < 1e-5)[0]
    KH = max(8, int(below[0])) if below.size else KH_MAX
    NB = (129 + KH + 1) // 2 * 2
    _CFG["KH"], _CFG["NB"] = KH, NB

    h = h48[:KH]
    H = np.cumsum(h)
    c = {1: -phi[1], 2: -(1.0 + phi[0]), 3: 1.0}

    # shift-invariant band: w(r) = coeff of y_t in eps_n, r = n - t
    rtab = np.zeros(4096)
    for r in range(-3, KH - 1):
        v = 0.0
        for m in (1, 2, 3):
            k = r + m
            if 0 <= k < KH:
                v += c[m] * h[k]
        rtab[r + 2048] = v

    p_ = np.arange(P)[:, None]
    gpack = np.zeros((P, 2 * NB), np.float64)
    # G_band[p, j] = w(j - 3 - p); tile k feeds out cols [128k-3, 128k+126+KH)
    j_ = np.arange(NB)[None, :]
    gpack[:, NB:2 * NB] = rtab[np.clip(j_ - 3 - p_, -2048, 2047) + 2048]
    # G0: truncated-start weights for tile 0 (out cols [0,126+KH))
    n_ = np.arange(NB - 3)[None, :]
    G0 = np.zeros((P, NB - 3))
    for m in (1, 2, 3):
        k = n_ + m - p_
        valid = (k >= 0) & (k <= np.minimum(n_, KH - 1))
        G0 += c[m] * np.where(valid, h[np.clip(k, 0, KH - 1)], 0.0)
    gpack[:, 3:NB] = G0

    gpack = gpack.astype(np.float32)
    bias0 = (-mu * H[np.minimum(np.arange(512), KH - 1)]).astype(np.float32)
    bias_const = float(-mu * H[KH - 1])
    return gpack, np.broadcast_to(bias0.reshape(1, 512), (P, 512)).copy(), bias_const


def build_program(bias_const, reps=1, internal=False):
    import concourse.bacc as bacc
    import concourse.mybir as mybir
    from concourse.tile import TileContext

    f32 = mybir.dt.float32
    bf16 = mybir.dt.bfloat16
    i8 = mybir.dt.int8
    alu = mybir.AluOpType
    NB = _CFG["NB"]
    GW = 2 * NB
    # output int8 scale: device stores round(W*y * oscale); |W*y| <= B with
    # oscale = 127/B guarantees no saturation; host divides and adds bias
    oscale = float(_CFG.get("oscale", 16.0))
    # bf16 PSUM accumulation: half-bank windows -> 16 rotation slots and
    # 2x-mode evacuation (16-bit PSUM operand); one extra bf16 rounding
    pdt = bf16 if _CFG.get("psum16", False) else f32
    mode = _CFG.get("mode", "full")   # full | ld | st | ldst (perf decomposition)
    # DMA queue layout:
    #  a: loads alternate sync/scalar rings, stores on SWDGE (gpsimd)
    #  b: everything on the sync ring -> FIFO phase-separates loads/stores
    #  c: loads on sync, stores on scalar (direction per HWDGE ring)
    #  d: loads alternate rings, stores alternate SWDGE/scalar
    layout = _CFG.get("dma", "a")

    nc = bacc.Bacc()
    g_in = nc.declare_dram_parameter("gmats", [P, GW], f32, isOutput=False)
    # y arrives host-swizzled to the SBUF tile image: row 128*(2h+u)+p,
    # col 512*kk+c  <->  y^T[2048u+128kk+p, 512h+c]; 16KB contiguous lines
    if internal:
        yio = nc.declare_dram_parameter("yio", [1, 4], f32, isOutput=True)
        y_in = nc.dram_tensor("ydr", [4 * P, 8192], bf16)
        out = nc.dram_tensor("odr", [B_SH, T_OUT], i8)
    else:
        y_in = nc.declare_dram_parameter("y", [4 * P, 8192], bf16, isOutput=False)
        out = nc.declare_dram_parameter("out", [B_SH, T_OUT], i8, isOutput=True)

    with TileContext(nc) as tc:
        with (
            tc.tile_pool(name="consts", bufs=1) as cpool,
            tc.tile_pool(name="ld", bufs=(3 if _CFG.get("ld4") else 4)) as ldpool,
            tc.tile_pool(name="st", bufs=3) as stpool,
            tc.tile_pool(name="pacc",
                         bufs=(16 if _CFG.get("psum16", False) else 8),
                         space="PSUM") as pacc,
        ):
            gr = cpool.tile([P, GW], bf16)
            nc.gpsimd.dma_start(out=gr[:], in_=g_in[:])   # SWDGE f32->bf16 cast
            g0 = gr[:, 0:NB]                  # tile-0 band (3 leading zero cols)
            gb = gr[:, NB:NB + NB]            # interior band [0:NB)
            gb_close = gr[:, NB:NB + 128]     # closing slice [0:128)
            gb_spill = gr[:, NB + 128:NB + NB]  # spill slice [128:NB)

            if internal:
                zf = cpool.tile([P, 8192], bf16)
                nc.vector.memset(zf[:], 0.0)
                for i in range(4):
                    nc.sync.dma_start(out=y_in[i * P:(i + 1) * P, :], in_=zf[:])

            # store group u covers windows 4u..4u+3; per-b width and the
            # SBUF col offset of window j's evacuation within the group tile
            STW = (2045, 2050)                # g0: 509+3*512, g1: 4*512+2 zeros

            def st_off(j, b):
                u, wg = j // 4, j % 4
                if u == 0:
                    return STW[0] * b + (0 if wg == 0 else 509 + 512 * (wg - 1))
                return STW[1] * b + 512 * wg

            def ld_eng(u):
                if layout in ("b", "c"):
                    return nc.sync
                return nc.sync if u % 2 == 0 else nc.scalar

            def st_eng(idx):
                if layout == "b":
                    return nc.sync
                if layout == "c":
                    return nc.scalar
                if layout == "d":
                    return nc.gpsimd if idx % 2 == 0 else nc.scalar
                return nc.gpsimd

            if mode in ("st", "ldst"):
                stz = [cpool.tile([P, 4 * STW[1]], i8, name=f"stz{i}")
                       for i in range(2)]
                for z in stz:
                    nc.vector.memset(z[:], 0.0)

            def body_strip():
                for h in range(2):
                    if mode in ("ld", "ldst"):
                        for u in range(2):
                            ld = ldpool.tile([P, 8192], bf16, tag="ld")
                            eng = nc.sync if u % 2 == 0 else nc.scalar
                            g = 2 * h + u
                            eng.dma_start(
                                out=ld[:], in_=y_in[P * g:P * (g + 1), :])
                    if mode in ("st", "ldst"):
                        for u2 in range(2):
                            c0 = 0 if u2 == 0 else 2045
                            c1 = 2045 if u2 == 0 else 4095
                            nc.gpsimd.dma_start(
                                out=out[512 * h:512 * (h + 1), c0:c1]
                                .rearrange("(k p) c -> p k c", p=P),
                                in_=stz[u2][:, 0:4 * (c1 - c0)])

            def body():
                if mode != "full":
                    body_strip()
                    return
                for h in range(2):            # batch halves: rows [512h, 512h+512)
                    wincur = [None] * 4
                    winnext = [None] * 4
                    st = None
                    if _CFG.get("ld4"):
                        # one 4MB load per half (16KB lines x2 per partition)
                        ld = ldpool.tile([P, 16384], bf16, tag="ld")
                        lds = [ld[:, 0:8192], ld[:, 8192:16384]]
                        eng = nc.sync if h == 0 else nc.scalar
                        eng.dma_start(
                            out=ld[:],
                            in_=y_in[256 * h:256 * (h + 1), :]
                            .rearrange("(k p) c -> p k c", p=P))
                    else:
                        lds = []
                        for u in range(2):    # 2MB load groups: tiles 16u..16u+15
                            ld = ldpool.tile([P, 8192], bf16, tag="ld")
                            lds.append(ld)
                            g = 2 * h + u
                            ld_eng(u).dma_start(
                                out=ld[:], in_=y_in[P * g:P * (g + 1), :])
                    for j in range(8):        # output window index
                        u = j // 4
                        for k4 in range(4):
                            k = 4 * j + k4
                            for b in range(4):
                                lo = 512 * (k - 16 * u) + 128 * b
                                lhs = lds[u][:, lo:lo + 128]
                                if k == 0:
                                    pw = pacc.tile([P, 512], pdt, tag="acc")
                                    wincur[b] = pw
                                    nc.tensor.matmul(pw[:, 0:NB], lhs, g0,
                                                     start=True, stop=False)
                                elif k4 < 3:
                                    nc.tensor.matmul(
                                        wincur[b][:, 128 * k4:128 * k4 + NB],
                                        lhs, gb, start=False, stop=False)
                                else:
                                    nc.tensor.matmul(wincur[b][:, 384:512], lhs,
                                                     gb_close, start=False, stop=True)
                                    if k < 31:
                                        pw = pacc.tile([P, 512], pdt, tag="acc")
                                        winnext[b] = pw
                                        nc.tensor.matmul(pw[:, 0:NB - 128], lhs,
                                                         gb_spill,
                                                         start=True, stop=False)
                                    # evacuate window j for this b-block: plain
                                    # PSUM->SBUF bf16 copy (bias added on host)
                                    if b == 0 and j % 4 == 0:
                                        st = stpool.tile([P, 4 * STW[1]], i8,
                                                         tag="st")
                                    o0 = st_off(j, b)
                                    src = (wincur[b][:, 3:512] if j == 0
                                           else wincur[b][:])
                                    dst = st[:, o0:o0 + (509 if j == 0 else 512)]
                                    if j == 7:
                                        nc.vector.memset(
                                            st[:, o0 + 512:o0 + 514], 0.0)
                                    # scaled f32 PSUM -> int8 SBUF evacuation,
                                    # split evenly DVE/ACT (24/40 split measured worse)
                                    ev = _CFG.get("evac", "mix")
                                    if ev == "dve" or (ev == "mix" and b % 2 == 0):
                                        nc.vector.tensor_scalar(
                                            out=dst, in0=src,
                                            scalar1=oscale, scalar2=0.0,
                                            op0=alu.mult, op1=alu.add)
                                    else:
                                        nc.scalar.activation(
                                            out=dst, in_=src,
                                            func=mybir.ActivationFunctionType.Copy,
                                            bias=0.0, scale=oscale)
                                    wincur[b] = winnext[b]
                        if j % 4 == 3:        # store the 4-window group
                            u2 = j // 4
                            c0 = 0 if u2 == 0 else 2045
                            c1 = 2045 if u2 == 0 else 4095
                            st_eng(2 * h + u2).dma_start(
                                out=out[512 * h:512 * (h + 1), c0:c1]
                                .rearrange("(k p) c -> p k c", p=P),
                                in_=st[:, 0:4 * (c1 - c0)])

            # reps = TOTAL body count; unrolling 2 bodies per For_i iteration
            # removes ~3.7us/rep of loop-boundary serialization (measured)
            unroll = _CFG.get("unroll", 2)
            if reps == 1:
                body()
            else:
                if reps % unroll:
                    unroll = 1
                with tc.For_i(0, reps // unroll, 1) as _r:
                    for _ in range(unroll):
                        body()
            if internal:
                nc.gpsimd.dma_start(out=yio[:], in_=gr[0:1, 0:4])
    nc.finalize()
    return nc


def kernel(y, phi, theta, mu):
    import ml_dtypes
    from concourse.bass_utils import run_bass_kernel_spmd

    y = np.asarray(y, np.float32)
    assert y.shape == (B_FULL, T), y.shape
    gmats, bias0, bias_const = host_constants(phi, theta, mu)

    # int8 output scale: rigorous no-saturation bound |W*y| <= sum|w|*max|y|
    NB = _CFG["NB"]
    b_w = float(np.abs(gmats[0, NB:2 * NB]).sum())
    bound = b_w * float(np.abs(y).max()) * 1.02 + 1e-6
    oscale = 127.0 / bound
    _CFG["oscale"] = oscale

    nc = build_program(bias_const)
    in_maps = []
    for c in range(N_CORES):
        yc = y[c * B_SH:(c + 1) * B_SH].astype(ml_dtypes.bfloat16)
        # swizzle to the SBUF tile image: [h, c, u, kk, p] -> [h, u, p, kk, c]
        ysw = np.ascontiguousarray(
            yc.reshape(2, 512, 2, 16, P).transpose(0, 2, 4, 3, 1)
        ).reshape(4 * P, 8192)
        in_maps.append({"y": ysw, "gmats": gmats})
    res = run_bass_kernel_spmd(nc, in_maps, list(range(N_CORES)))
    out = np.concatenate(
        [res.results[c]["out"].astype(np.float32) for c in range(N_CORES)],
        axis=0)
    out *= np.float32(1.0 / oscale)   # undo the int8 quantization scale
    # per-column drift bias applied host-side (device stores pure W*y)
    bias_full = np.full(T_OUT, bias_const, np.float32)
    bias_full[:512] = bias0[0]
    bias_full[N_EPS:] = 0.0
    out += bias_full[None, :]
    return out
